# revision 1
# baseline (speedup 1.0000x reference)
"""Modulated conv2d (StyleGAN-2 style, B=16 C=128 HxW=128x128 K=3) on 8 TRN2
NeuronCores, data-parallel over batch (2 samples/core), ~150 us HW time.

Per core:
  1. style:  s[i,n] = (mod_w @ w_n) + mod_b + 1          (PE, K=512 via 4 k-tiles)
  2. wmod[i,t*C+o] = wT[i,t*C+o] * s[i]  -> bf16         (DVE per-partition scalar)
  3. dsq[o]  = sum_{i,t} wmod^2   via 9 accumulated matmuls with ones-vector rhs
  4. d[o]    = 1/sqrt(dsq + 1e-8)                        (ACT sqrt + DVE recip)
  5. conv:   x zero-padded to 130x130 on host, bf16; per 4-row output block,
     9 tap matmuls (K=C_in, M=C_out, N=512) accumulate fp32 in PSUM; the tap
     shift is a strided 3D rhs view into the padded image — no im2col.
  6. evict:  out = psum * d[o]                           (DVE tensor_scalar)
  7. DMA out (ACT-issued HWDGE), 12 staging buffers, 6 rotating PSUM banks.

The demod matmuls are interleaved into the first conv blocks so the
PE->ACT->DVE d-chain completes while PE streams; x arrives in 4 chunks with
a small first chunk so the conv can start early; all scalar params ride in
one packed [128, 1675] tensor (2 DMAs) to avoid serial small-DMA latency.

Raw Bass with manual semaphores: this toolchain's walrus accepts only ONE
sync-wait command per instruction, so Tile's auto-sync output does not
compile; explicit single-wait `wait_ge` instructions do. Every engine-pair
dependency (including same-engine RAW, which the hardware does not order)
is guarded by an explicit semaphore.

Numerics: bf16 operands, fp32 accumulation; max rel err vs the fp32 jax
reference ~2.3e-3. Set PRECISION = "f32r" for the float32r variant
(11-bit mantissa, rel err ~1.4e-4, ~10 us slower).
"""

import sys

sys.path.insert(0, "/opt/trn_rl_repo")

import numpy as np

import concourse.bass as bass
from concourse import mybir
from concourse.bass_utils import run_bass_kernel_spmd

B, C, H, W, KS, WD = 16, 128, 128, 128, 3, 512
NCORES = 8
SPC = B // NCORES          # samples per core = 2
HP = H + 2                 # padded height/width = 130
NT = KS * KS               # 9 taps
KT = WD // 128             # 4 k-tiles for the style matmul
PRECISION = "bf16"         # "bf16" (N=1024, FWL) or "f32r" (N=512, 11-bit mantissa)

R = 4                      # output rows per conv block (N = R*W = 512; PSUM bank cap)
NPS = 6                    # rotating conv PSUM banks
NOB = 12                   # output staging buffers
NB = H // R                # conv blocks per sample
CHUNK_BNDS = [0, 10, 50, 90, 130]   # x DMA chunk row boundaries (padded rows)


def _chunk_of_block(b):
    """First x chunk that covers padded rows needed by output block b."""
    need = R * b + R + 1
    for c in range(len(CHUNK_BNDS) - 1):
        if need < CHUNK_BNDS[c + 1]:
            return c
    raise AssertionError


F32 = mybir.dt.float32
F32R = mybir.dt.float32r
BF16 = mybir.dt.bfloat16
ADD = mybir.AluOpType.add
MULT = mybir.AluOpType.mult
SQRT = mybir.ActivationFunctionType.Sqrt


def round_fp32r(a):
    """Round fp32 array to fp32r (1s + 8e + 11m stored in top 20 bits, RNE)."""
    u = np.ascontiguousarray(a, np.float32).view(np.uint32)
    lower = u & np.uint32(0xFFF)
    keep_lsb = (u >> np.uint32(12)) & np.uint32(1)
    add = (lower > 0x800) | ((lower == 0x800) & (keep_lsb == 1))
    ru = (u & np.uint32(0xFFFFF000)) + (add.astype(np.uint32) << np.uint32(12))
    return ru.view(np.float32)


def build_program():
    nc = bass.Bass(trn_type="TRN2", target_bir_lowering=False, debug=False)
    xdt = BF16 if PRECISION == "bf16" else F32R

    NPS_R = KT * C + KT * SPC + 3            # f32r style: mwT | wvec | ones(x2) | modb
    NPF = 1 + NT * C                         # f32 params: eps | wT
    xpad_d = nc.dram_tensor("xpad", [SPC, C, HP, HP], xdt, kind="ExternalInput").ap()
    params_r_d = nc.dram_tensor("params_r", [C, NPS_R], F32R, kind="ExternalInput").ap()
    params_f_d = nc.dram_tensor("params_f", [C, NPF], F32, kind="ExternalInput").ap()
    y_d = nc.dram_tensor("y", [SPC, C, H, W], F32, kind="ExternalOutput").ap()

    xs = nc.alloc_sbuf_tensor("xs", [C, SPC, HP, HP], xdt).ap()
    params_r = nc.alloc_sbuf_tensor("params_r_sb", [C, NPS_R], F32R).ap()
    mwTs = params_r[:, 0 : KT * C].rearrange("p (k c) -> p k c", k=KT)
    wvecTs = params_r[:, KT * C : KT * C + KT * SPC].rearrange(
        "p (k c) -> p k c", k=KT)
    oness = params_r[:, NPS_R - 3 : NPS_R - 1]  # [C, 2] ones (fp32r MM needs even N)
    modbs = params_r[:, NPS_R - 1 : NPS_R].bitcast(F32)
    params_f = nc.alloc_sbuf_tensor("params_f_sb", [C, NPF], F32).ap()
    epss = params_f[:, 0:1]
    wTs = params_f[:, 1 : 1 + NT * C]
    wmod = nc.alloc_sbuf_tensor("wmod", [C, SPC, NT * C], xdt).ap()
    sq = nc.alloc_sbuf_tensor("sq", [C, SPC, NT * C], F32R).ap()
    outsb = nc.alloc_sbuf_tensor("outsb", [C, NOB, R * W], F32).ap()
    s_sb = nc.alloc_sbuf_tensor("s_sb", [C, SPC], F32).ap()
    dsr = nc.alloc_sbuf_tensor("dsr", [C, SPC], F32).ap()
    dcol = nc.alloc_sbuf_tensor("dcol", [C, SPC], F32).ap()

    cps = [nc.alloc_psum_tensor(f"cps{j}", [C, R * W], F32).ap() for j in range(NPS)]
    sps = nc.alloc_psum_tensor("sps", [C, SPC], F32).ap()
    dps = nc.alloc_psum_tensor("dps", [C, 2 * SPC], F32).ap()

    sem_x = [nc.alloc_semaphore(f"sx{i}") for i in range(SPC * 4)]
    sem_dma_param = nc.alloc_semaphore("sdma_param")   # style params (-> 16)
    sem_dma_wt = nc.alloc_semaphore("sdma_wt")         # wT (-> 16)
    sem_pe_style = nc.alloc_semaphore("pe_style")
    sem_dve_sq = nc.alloc_semaphore("dve_sq")
    sem_dve_w = nc.alloc_semaphore("dve_w")
    sem_pe_dcol = nc.alloc_semaphore("pe_dcol")
    sem_act_sqrt = nc.alloc_semaphore("act_sqrt")
    sem_pe_blk = nc.alloc_semaphore("pe_blk")
    sem_dve_evict = nc.alloc_semaphore("dve_evict")
    sem_dve_self = nc.alloc_semaphore("dve_self")
    sem_od = [nc.alloc_semaphore(f"sod{j}") for j in range(NOB)]

    with nc.Block() as blk:

        @blk.sync
        def _(eng):
            def xchunk(s, ci):
                r0, r1 = CHUNK_BNDS[ci], CHUNK_BNDS[ci + 1]
                eng.dma_start(
                    out=xs[:, s : s + 1, r0:r1, :],
                    in_=xpad_d[s : s + 1, :, r0:r1, :],
                ).then_inc(sem_x[4 * s + ci], 16)

            xchunk(0, 0)
            eng.dma_start(out=params_f, in_=params_f_d).then_inc(sem_dma_wt, 16)
            for ci in range(1, 4):
                xchunk(0, ci)
            for ci in range(4):
                xchunk(1, ci)

        @blk.tensor
        def _(eng):
            # style matmul: sps[i, n] = sum_d mod_w[i, d] * w[n, d]
            eng.wait_ge(sem_dma_param, 16)
            for kt in range(KT):
                inst = eng.matmul(
                    out=sps,
                    lhsT=mwTs[:, kt : kt + 1, :],
                    rhs=wvecTs[:, kt : kt + 1, :],
                    start=(kt == 0),
                    stop=(kt == KT - 1),
                )
            inst.then_inc(sem_pe_style, 1)

            def demod(s):
                # dps[o, s] = sum_{i, t} wmod[i, s, t*C+o]^2
                eng.wait_ge(sem_dve_sq, s + 1)
                if s >= 1:
                    eng.wait_ge(sem_act_sqrt, s)   # dps bank WAR vs ACT read
                for t in range(NT):
                    inst = eng.matmul(
                        out=dps[:, 2 * s : 2 * s + 2],
                        lhsT=sq[:, s : s + 1, t * C : (t + 1) * C],
                        rhs=oness,
                        start=(t == 0),
                        stop=(t == NT - 1),
                    )
                inst.then_inc(sem_pe_dcol, 1)

            def conv_block(s, b, gb):
                if b == 0 or _chunk_of_block(b) != _chunk_of_block(b - 1):
                    eng.wait_ge(sem_x[4 * s + _chunk_of_block(b)], 16)
                if gb >= NPS and (gb - NPS) % 4 == 0:
                    # covers bank reuse for blocks gb..gb+3 (reuse distance NPS)
                    eng.wait_ge(sem_dve_evict, gb - NPS + 4)
                for kh in range(KS):
                    for kw in range(KS):
                        t = kh * KS + kw
                        inst = eng.matmul(
                            out=cps[gb % NPS],
                            lhsT=wmod[:, s : s + 1, t * C : (t + 1) * C],
                            rhs=xs[:, s : s + 1, R * b + kh : R * b + kh + R,
                                   kw : kw + W],
                            start=(t == 0),
                            stop=(t == NT - 1),
                        )
                inst.then_inc(sem_pe_blk, 1)

            # interleave the demod matmuls into the first conv blocks so the
            # d-chain (PE->ACT->DVE) completes while PE streams early blocks
            eng.wait_ge(sem_dve_w, 1)
            conv_block(0, 0, 0)
            demod(0)
            conv_block(0, 1, 1)
            conv_block(0, 2, 2)
            demod(1)
            for b in range(3, NB):
                conv_block(0, b, b)
            eng.wait_ge(sem_dve_w, 2)
            for b in range(NB):
                conv_block(1, b, NB + b)

        @blk.vector
        def _(eng):
            eng.wait_ge(sem_pe_style, 1)
            eng.tensor_scalar(s_sb, sps, modbs, 1.0, ADD, ADD).then_inc(sem_dve_self, 1)
            eng.wait_ge(sem_dma_wt, 16)  # wT landed
            nself = 1
            for s in range(SPC):
                eng.wait_ge(sem_dve_self, nself)  # s_sb ready (same-engine RAW)
                eng.tensor_scalar(wmod[:, s : s + 1, :], wTs, s_sb[:, s : s + 1],
                                  None, MULT).then_inc(sem_dve_w, 1)
                eng.wait_ge(sem_dve_w, s + 1)     # wmod ready (same-engine RAW)
                wm_in = (wmod[:, s : s + 1, :] if PRECISION == "bf16"
                         else wmod[:, s : s + 1, :].bitcast(F32))
                eng.tensor_tensor(sq[:, s : s + 1, :], wm_in,
                                  wm_in, MULT).then_inc(sem_dve_sq, 1)
            for s in range(SPC):
                eng.wait_ge(sem_act_sqrt, s + 1)
                eng.reciprocal(dcol[:, s : s + 1], dsr[:, s : s + 1]).then_inc(
                    sem_dve_self, 1)
                nself += 1
            eng.wait_ge(sem_dve_self, nself)  # dcol ready for evictions
            # evictions: out = psum * d[o]
            for gb in range(SPC * NB):
                s = gb // NB
                eng.wait_ge(sem_pe_blk, gb + 1)
                if gb >= NOB:
                    eng.wait_ge(sem_od[gb % NOB], 16 * (gb // NOB))
                eng.tensor_scalar(outsb[:, gb % NOB : gb % NOB + 1, :],
                                  cps[gb % NPS], dcol[:, s : s + 1],
                                  None, MULT).then_inc(sem_dve_evict, 1)

        @blk.scalar
        def _(eng):
            # critical style-param DMA alone on ACT's HWDGE queue
            eng.dma_start(out=params_r, in_=params_r_d).then_inc(sem_dma_param, 16)
            for s in range(SPC):
                eng.wait_ge(sem_pe_dcol, s + 1)
                eng.activation(dsr[:, s : s + 1], dps[:, 2 * s : 2 * s + 1], SQRT,
                               bias=epss).then_inc(sem_act_sqrt, 1)
            # output DMAs (ACT is a HWDGE engine)
            for gb in range(SPC * NB):
                s, b = gb // NB, gb % NB
                eng.wait_ge(sem_dve_evict, gb + 1)
                eng.dma_start(
                    out=y_d[s : s + 1, :, R * b : R * b + R, :],
                    in_=outsb[:, gb % NOB : gb % NOB + 1, :],
                ).then_inc(sem_od[gb % NOB], 16)

    return nc


def _host_prep(x, w, weight, mod_w, mod_b):
    f = np.float32
    x = np.asarray(x, f)
    w = np.asarray(w, f)
    weight = np.asarray(weight, f)
    mod_w = np.asarray(mod_w, f)
    mod_b = np.asarray(mod_b, f)

    if PRECISION == "bf16":
        import ml_dtypes
        xpad = np.zeros((B, C, HP, HP), ml_dtypes.bfloat16)
        xpad[:, :, 1 : H + 1, 1 : W + 1] = x.astype(ml_dtypes.bfloat16)
    else:
        xpad = np.zeros((B, C, HP, HP), f)
        xpad[:, :, 1 : H + 1, 1 : W + 1] = round_fp32r(x)

    # params_r (f32r): mwT | wvecT | ones ; params_f (f32): modb | eps | wT
    NPS_R = KT * C + KT * SPC + 3
    NPF = 1 + NT * C
    # wT[i, t*C + o] = weight[o, i, kh, kw],  t = kh*3 + kw
    wT = weight.transpose(1, 2, 3, 0).reshape(C, NT * C)
    # mwT[d_lo, kt, i] = mod_w[i, kt*128 + d_lo]
    mwT = mod_w.T.reshape(KT, 128, C).transpose(1, 0, 2).reshape(C, KT * C)
    base_r = np.empty((C, NPS_R), f)
    base_r[:, : KT * C] = round_fp32r(mwT)
    base_r[:, NPS_R - 3 : NPS_R - 1] = 1.0
    base_r[:, NPS_R - 1] = round_fp32r(mod_b)
    base_f = np.empty((C, NPF), f)
    base_f[:, 0] = 1e-8
    base_f[:, 1:] = wT

    in_maps = []
    for core in range(NCORES):
        s0 = SPC * core
        # wvecT[d_lo, kt, n] = w[s0 + n, kt*128 + d_lo]
        wvecT = (w[s0 : s0 + SPC].T.reshape(KT, 128, SPC)
                 .transpose(1, 0, 2).reshape(C, KT * SPC))
        pr = base_r.copy()
        pr[:, KT * C : KT * C + KT * SPC] = round_fp32r(wvecT)
        in_maps.append({
            "xpad": np.ascontiguousarray(xpad[s0 : s0 + SPC]),
            "params_r": pr,
            "params_f": base_f,
        })
    return in_maps


_cached = {}


def kernel(x, w, weight, mod_w, mod_b):
    if "nc" not in _cached:
        _cached["nc"] = build_program()
    nc = _cached["nc"]
    in_maps = _host_prep(x, w, weight, mod_w, mod_b)
    res = run_bass_kernel_spmd(nc, in_maps, list(range(NCORES)))
    return np.concatenate([res.results[i]["y"] for i in range(NCORES)], axis=0)


if __name__ == "__main__":
    from concourse.bass_utils import compile_bass_kernel
    import tempfile

    nc = build_program()
    d = tempfile.mkdtemp()
    neff = compile_bass_kernel(nc, d)
    print("compiled OK:", neff)



# revision 2
# speedup vs baseline: 1.1640x; 1.1640x over previous
"""Modulated conv2d (StyleGAN-2 style, B=16 C=128 HxW=128x128 K=3) on 8 TRN2
NeuronCores, data-parallel over batch (2 samples/core), via 1D Winograd
F(2,3) along W.

All input-side transforms are pure functions of the inputs and run on HOST:
  s[b,i]   = Linear(w)+1 (style), folded into x:  xt = s * x
  d[b,o]   = demod rsqrt(sum((weight*s)^2)+eps), folded into the weights
  U[xi]    = 1D Winograd input transform of padded xt (4 tensors, bf16):
               u0=xe[m]-xe[m+1], u1=xo[m]+xe[m+1], u2=xe[m+1]-xo[m],
               u3=xo[m]-xo[m+1]        (xe/xo = even/odd padded columns)
  g[xi,kh] = G-transformed demodulated base weight (per sample, bf16):
               g0=W0, g1=(W0+W1+W2)/2, g2=(W0-W1+W2)/2, g3=W2   (kw taps)

Device work per 8-row output block (32 blocks/core, PSUM-bank sized N=512):
  PE:     12 matmuls (4 xi-groups x 3 kh, K=C_in=128) -> M0..M3 in 4 banks
  ACT:    copy M1,M2,M3 from PSUM to SBUF (m1s,m2s,m3s)
  DVE:    u=m1s+m2s, v=m1s-m2s, ye=(M0+u) -> bf16   (even output columns)
  GPSIMD: yo=(v-m3s) -> bf16                        (odd output columns)
  Winograd identity: ye = M0+M1+M2, yo = M1-M2-M3.
Even/odd column planes DMA out as separate bf16 tensors; host interleaves.

This cuts PE streaming cycles 1.5x vs direct conv (12xN=512 per 1024
outputs vs 18xN=512): PE ~83us vs the ~125us direct-conv floor. GPSIMD
cannot read PSUM (hardware rule: max one PSUM operand per vector op), hence
the ACT copies. Weight loads (12/block) hide under the 216ns matmul streams.

Raw Bass with manual semaphores (single-wait rule; every cross-engine and
PSUM/SBUF WAR dependency guarded). Numerics: bf16 operands, fp32 PSUM
accumulation and output transform, bf16 output; rel err ~4e-3 vs fp32 ref.
"""

import sys

sys.path.insert(0, "/opt/trn_rl_repo")

import numpy as np

import concourse.bass as bass
from concourse import mybir
from concourse.bass_utils import run_bass_kernel_spmd

B, C, H, W, KS, WD = 16, 128, 128, 128, 3, 512
NCORES = 8
SPC = B // NCORES          # samples per core = 2
HP = H + 2                 # padded rows = 130
M = W // 2                 # output column pairs = 64
XI = 4                     # winograd components
RB = 8                     # output rows per block (N = RB*M = 512, one bank)
NBS = H // RB              # blocks per sample = 16
NB = SPC * NBS             # blocks per core = 32
NSLOT = 3                  # output staging slots (4 blocks each)
GRPB = 4                   # blocks per output DMA group
NGRP = NB // GRPB          # 8 output DMA groups
CHUNK_BNDS = [0, 10, 18, 66, 114, 130]   # U DMA row chunks
NCH = len(CHUNK_BNDS) - 1

F32 = mybir.dt.float32
BF16 = mybir.dt.bfloat16
ADD = mybir.AluOpType.add
SUB = mybir.AluOpType.subtract
COPY = mybir.ActivationFunctionType.Copy


def _chunk_of_block(b):
    need = RB * b + RB + 1
    for c in range(NCH):
        if need < CHUNK_BNDS[c + 1]:
            return c
    raise AssertionError


def build_program():
    nc = bass.Bass(trn_type="TRN2", target_bir_lowering=False, debug=False)

    # DRAM. U rows are flattened (s, xi, c) -> partition-sliced 3D APs.
    u_d = nc.dram_tensor("u", [SPC * XI * C, HP, M], BF16, kind="ExternalInput").ap()
    g_d = nc.dram_tensor("g", [C, SPC * 12 * C], BF16, kind="ExternalInput").ap()
    ye_d = nc.dram_tensor("ye", [SPC * C, H, M], BF16, kind="ExternalOutput").ap()
    yo_d = nc.dram_tensor("yo", [SPC * C, H, M], BF16, kind="ExternalOutput").ap()

    # SBUF (per partition: 130KB U + 6KB g + 12KB m + 8KB uv + 24KB ost)
    u_sb = nc.alloc_sbuf_tensor("u_sb", [C, SPC * XI * HP, M], BF16).ap()
    g_sb = nc.alloc_sbuf_tensor("g_sb", [C, SPC * 12 * C], BF16).ap()
    m1s = nc.alloc_sbuf_tensor("m1s", [C, 2 * 512], F32).ap()
    m2s = nc.alloc_sbuf_tensor("m2s", [C, 2 * 512], F32).ap()
    m3s = nc.alloc_sbuf_tensor("m3s", [C, 2 * 512], F32).ap()
    uv = nc.alloc_sbuf_tensor("uv", [C, 2 * 2 * 512], F32).ap()
    ost = nc.alloc_sbuf_tensor("ost", [C, NSLOT * 2 * GRPB * 512], BF16).ap()

    pb = [nc.alloc_psum_tensor(f"pb{j}", [C, 512], F32).ap() for j in range(8)]

    s_u = [nc.alloc_semaphore(f"su{i}") for i in range(SPC * NCH)]
    s_w = [nc.alloc_semaphore(f"sw{i}") for i in range(SPC)]
    s_pe = nc.alloc_semaphore("s_pe")      # +1 per xi-group (4/block)
    s_ac = nc.alloc_semaphore("s_ac")      # +1 per ACT copy (3/block)
    s_vv = nc.alloc_semaphore("s_vv")      # +1 per DVE v
    s_vy = nc.alloc_semaphore("s_vy")      # +1 per DVE ye
    s_gp = nc.alloc_semaphore("s_gp")      # +1 per GPSIMD yo
    s_od = [nc.alloc_semaphore(f"sod{i}") for i in range(NSLOT)]

    def urow(s, xi, r):
        return (s * XI + xi) * HP + r

    def gcol(s, xi, kh):
        return (s * 12 + 3 * xi + kh) * C

    with nc.Block() as blk:

        @blk.sync
        def _(eng):
            def uchunk(s, ci):
                r0, r1 = CHUNK_BNDS[ci], CHUNK_BNDS[ci + 1]
                for xi in range(XI):
                    eng.dma_start(
                        out=u_sb[:, urow(s, xi, r0) : urow(s, xi, r1), :],
                        in_=u_d[(s * XI + xi) * C : (s * XI + xi + 1) * C, r0:r1, :],
                    ).then_inc(s_u[s * NCH + ci], 16)

            eng.dma_start(out=g_sb[:, 0 : 12 * C], in_=g_d[:, 0 : 12 * C]).then_inc(
                s_w[0], 16)
            uchunk(0, 0)
            uchunk(0, 1)
            eng.dma_start(out=g_sb[:, 12 * C :], in_=g_d[:, 12 * C :]).then_inc(
                s_w[1], 16)
            for ci in range(2, NCH):
                uchunk(0, ci)
            for ci in range(NCH):
                uchunk(1, ci)
            # output DMAs (in-order queue; all input issues precede these)
            for grp in range(NGRP):
                s, r0, slot = grp // (NGRP // SPC), RB * GRPB * (grp % (NGRP // SPC)), grp % NSLOT
                eng.wait_ge(s_vy, GRPB * grp + GRPB)
                eng.dma_start(
                    out=ye_d[s * C : (s + 1) * C, r0 : r0 + RB * GRPB, :],
                    in_=ost[:, (slot * 2 + 0) * 2048 : (slot * 2 + 1) * 2048],
                ).then_inc(s_od[slot], 16)
                eng.wait_ge(s_gp, GRPB * grp + GRPB)
                eng.dma_start(
                    out=yo_d[s * C : (s + 1) * C, r0 : r0 + RB * GRPB, :],
                    in_=ost[:, (slot * 2 + 1) * 2048 : (slot * 2 + 2) * 2048],
                ).then_inc(s_od[slot], 16)

        @blk.tensor
        def _(eng):
            eng.wait_ge(s_w[0], 16)
            for gb in range(NB):
                s, b = gb // NBS, gb % NBS
                if gb == NBS:
                    eng.wait_ge(s_w[1], 16)
                c = _chunk_of_block(b)
                if b == 0 or c != _chunk_of_block(b - 1):
                    eng.wait_ge(s_u[s * NCH + c], 16 * XI)
                if gb >= 2:
                    eng.wait_ge(s_ac, 3 * (gb - 2) + 3)   # M1..M3 of gb-2 read
                    eng.wait_ge(s_vy, gb - 1)             # M0 of gb-2 read
                par = gb % 2
                for xi in range(XI):
                    for kh in range(KS):
                        inst = eng.matmul(
                            out=pb[par * 4 + xi],
                            lhsT=g_sb[:, gcol(s, xi, kh) : gcol(s, xi, kh) + C],
                            rhs=u_sb[:, urow(s, xi, RB * b + kh) : urow(s, xi, RB * b + kh + RB), :],
                            start=(kh == 0),
                            stop=(kh == KS - 1),
                        )
                    inst.then_inc(s_pe, 1)

        @blk.scalar
        def _(eng):
            for gb in range(NB):
                par = gb % 2
                if gb >= 2:
                    eng.wait_ge(s_vv, gb - 1)   # m1s/m2s[par] consumers done
                    eng.wait_ge(s_gp, gb - 1)   # m3s[par] consumer done
                eng.wait_ge(s_pe, 4 * gb + 2)
                eng.activation(m1s[:, par * 512 : par * 512 + 512],
                               pb[par * 4 + 1], COPY).then_inc(s_ac, 1)
                eng.wait_ge(s_pe, 4 * gb + 3)
                eng.activation(m2s[:, par * 512 : par * 512 + 512],
                               pb[par * 4 + 2], COPY).then_inc(s_ac, 1)
                eng.wait_ge(s_pe, 4 * gb + 4)
                eng.activation(m3s[:, par * 512 : par * 512 + 512],
                               pb[par * 4 + 3], COPY).then_inc(s_ac, 1)

        @blk.vector
        def _(eng):
            for gb in range(NB):
                par, grp, j = gb % 2, gb // GRPB, gb % GRPB
                slot = grp % NSLOT
                eng.wait_ge(s_ac, 3 * gb + 2)     # m1s, m2s ready
                if gb >= 2:
                    eng.wait_ge(s_gp, gb - 1)     # uv.v[par] consumer done
                if grp >= NSLOT and j == 0:
                    eng.wait_ge(s_od[slot], 32 * (grp // NSLOT))
                mp1 = m1s[:, par * 512 : par * 512 + 512]
                mp2 = m2s[:, par * 512 : par * 512 + 512]
                eng.tensor_tensor(uv[:, par * 1024 : par * 1024 + 512],
                                  mp1, mp2, ADD)
                eng.tensor_tensor(uv[:, par * 1024 + 512 : par * 1024 + 1024],
                                  mp1, mp2, SUB).then_inc(s_vv, 1)
                dst = (slot * 2 + 0) * 2048 + j * 512
                eng.tensor_tensor(ost[:, dst : dst + 512],
                                  pb[par * 4 + 0],
                                  uv[:, par * 1024 : par * 1024 + 512],
                                  ADD).then_inc(s_vy, 1)

        @blk.gpsimd
        def _(eng):
            for gb in range(NB):
                par, grp, j = gb % 2, gb // GRPB, gb % GRPB
                slot = grp % NSLOT
                eng.wait_ge(s_vv, gb + 1)         # v ready
                eng.wait_ge(s_ac, 3 * gb + 3)     # m3s ready
                if grp >= NSLOT and j == 0:
                    eng.wait_ge(s_od[slot], 32 * (grp // NSLOT))
                dst = (slot * 2 + 1) * 2048 + j * 512
                eng.tensor_tensor(ost[:, dst : dst + 512],
                                  uv[:, par * 1024 + 512 : par * 1024 + 1024],
                                  m3s[:, par * 512 : par * 512 + 512],
                                  SUB).then_inc(s_gp, 1)

    return nc


def _host_prep(x, w, weight, mod_w, mod_b):
    f = np.float32
    import ml_dtypes
    bf = ml_dtypes.bfloat16
    x = np.asarray(x, f)
    w = np.asarray(w, f)
    weight = np.asarray(weight, f)
    mod_w = np.asarray(mod_w, f)
    mod_b = np.asarray(mod_b, f)

    s_style = (w @ mod_w.T + mod_b) + 1.0                      # [B, C_in]
    a_sq = (weight ** 2).sum(axis=(2, 3))                      # [C_out, C_in]
    d = 1.0 / np.sqrt((s_style ** 2) @ a_sq.T + 1e-8)          # [B, C_out]

    # G-transformed demodulated weights (style folded into x instead)
    wd = weight[None] * d[:, :, None, None, None]              # [B, o, i, kh, kw]
    g0 = wd[..., 0]
    g1 = 0.5 * (wd[..., 0] + wd[..., 1] + wd[..., 2])
    g2 = 0.5 * (wd[..., 0] - wd[..., 1] + wd[..., 2])
    g3 = wd[..., 2]
    G = np.stack([g0, g1, g2, g3], axis=1)                     # [B, xi, o, i, kh]
    G = np.ascontiguousarray(G.transpose(0, 3, 1, 4, 2))       # [B, i, xi, kh, o]
    G = G.astype(bf)

    # style-modulated, padded input; even/odd columns; winograd transform
    xp = np.zeros((B, C, HP, HP), f)
    xp[:, :, 1 : H + 1, 1 : W + 1] = x * s_style[:, :, None, None]
    xe = xp[..., 0::2]
    xo = xp[..., 1::2]
    U = np.empty((B, XI, C, HP, M), f)
    U[:, 0] = xe[..., :M] - xe[..., 1:]
    U[:, 1] = xo[..., :M] + xe[..., 1:]
    U[:, 2] = xe[..., 1:] - xo[..., :M]
    U[:, 3] = xo[..., :M] - xo[..., 1:]
    U = U.astype(bf)

    in_maps = []
    for core in range(NCORES):
        s0 = SPC * core
        in_maps.append({
            "u": np.ascontiguousarray(U[s0 : s0 + SPC]).reshape(SPC * XI * C, HP, M),
            "g": np.ascontiguousarray(
                G[s0 : s0 + SPC].transpose(1, 0, 2, 3, 4)).reshape(C, SPC * 12 * C),
        })
    return in_maps


def _gather(res):
    y = np.empty((B, C, H, W), np.float32)
    for core in range(NCORES):
        ye = np.asarray(res.results[core]["ye"]).astype(np.float32).reshape(SPC, C, H, M)
        yo = np.asarray(res.results[core]["yo"]).astype(np.float32).reshape(SPC, C, H, M)
        for s in range(SPC):
            y[SPC * core + s, :, :, 0::2] = ye[s]
            y[SPC * core + s, :, :, 1::2] = yo[s]
    return y


_cached = {}


def kernel(x, w, weight, mod_w, mod_b):
    if "nc" not in _cached:
        _cached["nc"] = build_program()
    nc = _cached["nc"]
    in_maps = _host_prep(x, w, weight, mod_w, mod_b)
    res = run_bass_kernel_spmd(nc, in_maps, list(range(NCORES)))
    return _gather(res)


if __name__ == "__main__":
    from concourse.bass_utils import compile_bass_kernel
    import tempfile

    nc = build_program()
    d = tempfile.mkdtemp()
    neff = compile_bass_kernel(nc, d)
    print("compiled OK:", neff)


# revision 6
# speedup vs baseline: 1.2783x; 1.0982x over previous
"""Modulated conv2d (StyleGAN-2 style, B=16 C=128 HxW=128x128 K=3) on 8 TRN2
NeuronCores, data-parallel over batch (2 samples/core), via 1D Winograd
F(2,3) along W.

All input-side transforms are pure functions of the inputs and run on HOST:
  s[b,i]   = Linear(w)+1 (style), folded into x:  xt = s * x
  d[b,o]   = demod rsqrt(sum((weight*s)^2)+eps), folded into the weights
  U[xi]    = 1D Winograd input transform of padded xt (4 tensors, bf16):
               u0=xe[m]-xe[m+1], u1=xo[m]+xe[m+1], u2=xe[m+1]-xo[m],
               u3=xo[m]-xo[m+1]        (xe/xo = even/odd padded columns)
  g[xi,kh] = G-transformed demodulated base weight (per sample, bf16):
               g0=W0, g1=(W0+W1+W2)/2, g2=(W0-W1+W2)/2, g3=W2   (kw taps)

Device work per 8-row output block (32 blocks/core, PSUM-bank sized N=512):
  PE:     12 matmuls (4 xi-groups x 3 kh, K=C_in=128) -> M0..M3 in 4 banks
  ACT:    copy M1,M2,M3 from PSUM to SBUF (m1s,m2s,m3s)
  DVE:    u=m1s+m2s, v=m1s-m2s, ye=(M0+u) -> bf16   (even output columns)
  GPSIMD: yo=(v-m3s) -> bf16                        (odd output columns)
  Winograd identity: ye = M0+M1+M2, yo = M1-M2-M3.
Even/odd column planes DMA out as separate bf16 tensors; host interleaves.

This cuts PE streaming cycles 1.5x vs direct conv (12xN=512 per 1024
outputs vs 18xN=512): PE ~83us vs the ~125us direct-conv floor. GPSIMD
cannot read PSUM (hardware rule: max one PSUM operand per vector op), hence
the ACT copies. Weight loads (12/block) hide under the 216ns matmul streams.

Raw Bass with manual semaphores (single-wait rule; every cross-engine and
PSUM/SBUF WAR dependency guarded). Numerics: bf16 operands, fp32 PSUM
accumulation and output transform, bf16 output; rel err ~4e-3 vs fp32 ref.
"""

import sys

sys.path.insert(0, "/opt/trn_rl_repo")

import numpy as np

import concourse.bass as bass
from concourse import mybir
from concourse.bass_utils import run_bass_kernel_spmd

B, C, H, W, KS, WD = 16, 128, 128, 128, 3, 512
NCORES = 8
SPC = B // NCORES          # samples per core = 2
HP = H + 2                 # padded rows = 130
M = W // 2                 # output column pairs = 64
XI = 4                     # winograd components
RB = 8                     # output rows per block (N = RB*M = 512, one bank)
NBS = H // RB              # blocks per sample = 16
NB = SPC * NBS             # blocks per core = 32
NSLOT = 3                  # output staging slots (4 blocks each)
GRPB = 4                   # blocks per output DMA group
NGRP = NB // GRPB          # 8 output DMA groups
CHUNK_BNDS = [0, 10, 18, 66, 114, 130]   # U DMA row chunks
NCH = len(CHUNK_BNDS) - 1

F32 = mybir.dt.float32
BF16 = mybir.dt.bfloat16
ADD = mybir.AluOpType.add
SUB = mybir.AluOpType.subtract
COPY = mybir.ActivationFunctionType.Copy


def _chunk_of_block(b):
    need = RB * b + RB + 1
    for c in range(NCH):
        if need < CHUNK_BNDS[c + 1]:
            return c
    raise AssertionError


def build_program():
    nc = bass.Bass(trn_type="TRN2", target_bir_lowering=False, debug=False)

    # DRAM. U row layout [c, row, xi*M]: one DMA per (sample, row-chunk).
    u_d = nc.dram_tensor("u", [SPC * C, HP, XI * M], BF16, kind="ExternalInput").ap()
    g_d = nc.dram_tensor("g", [C, SPC * 12 * C], BF16, kind="ExternalInput").ap()
    ye_d = nc.dram_tensor("ye", [SPC * C, H, M], BF16, kind="ExternalOutput").ap()
    yo_d = nc.dram_tensor("yo", [SPC * C, H, M], BF16, kind="ExternalOutput").ap()

    # SBUF (per partition: 130KB U + 6KB g + 12KB m + 8KB uv + 24KB ost)
    u_sb = nc.alloc_sbuf_tensor("u_sb", [C, SPC * HP, XI * M], BF16).ap()
    g_sb = nc.alloc_sbuf_tensor("g_sb", [C, SPC * 12 * C], BF16).ap()
    wup = nc.alloc_sbuf_tensor("wup", [C, 640], BF16).ap()  # PE warmup scratch
    m1s = nc.alloc_sbuf_tensor("m1s", [C, 2 * 512], F32).ap()
    m2s = nc.alloc_sbuf_tensor("m2s", [C, 2 * 512], F32).ap()
    m3s = nc.alloc_sbuf_tensor("m3s", [C, 2 * 512], F32).ap()
    uv = nc.alloc_sbuf_tensor("uv", [C, 2 * 2 * 512], F32).ap()
    ost = nc.alloc_sbuf_tensor("ost", [C, NSLOT * 2 * GRPB * 512], BF16).ap()

    pb = [nc.alloc_psum_tensor(f"pb{j}", [C, 512], F32).ap() for j in range(8)]

    s_u = [nc.alloc_semaphore(f"su{i}") for i in range(SPC * NCH)]
    s_w = [nc.alloc_semaphore(f"sw{i}") for i in range(SPC)]
    s_pe = nc.alloc_semaphore("s_pe")      # +1 per xi-group (4/block)
    s_ac = nc.alloc_semaphore("s_ac")      # +1 per ACT copy (3/block)
    s_vv = nc.alloc_semaphore("s_vv")      # +1 per DVE v
    s_vy = nc.alloc_semaphore("s_vy")      # +1 per DVE ye
    s_gp = nc.alloc_semaphore("s_gp")      # +1 per GPSIMD yo
    s_od = [nc.alloc_semaphore(f"sod{i}") for i in range(NSLOT)]

    def gcol(s, xi, kh):
        return (s * 12 + 3 * xi + kh) * C

    with nc.Block() as blk:

        @blk.sync
        def _(eng):
            def uchunk(s, ci):
                r0, r1 = CHUNK_BNDS[ci], CHUNK_BNDS[ci + 1]
                eng.dma_start(
                    out=u_sb[:, s * HP + r0 : s * HP + r1, :],
                    in_=u_d[s * C : (s + 1) * C, r0:r1, :],
                ).then_inc(s_u[s * NCH + ci], 16)

            eng.dma_start(out=g_sb[:, 0 : 12 * C], in_=g_d[:, 0 : 12 * C]).then_inc(
                s_w[0], 16)
            uchunk(0, 0)
            uchunk(0, 1)
            eng.dma_start(out=g_sb[:, 12 * C :], in_=g_d[:, 12 * C :]).then_inc(
                s_w[1], 16)
            for ci in range(2, NCH):
                uchunk(0, ci)
            for ci in range(NCH):
                uchunk(1, ci)
            # output DMAs (in-order queue; all input issues precede these)
            for grp in range(NGRP):
                s, r0, slot = grp // (NGRP // SPC), RB * GRPB * (grp % (NGRP // SPC)), grp % NSLOT
                eng.wait_ge(s_vy, GRPB * grp + GRPB)
                eng.dma_start(
                    out=ye_d[s * C : (s + 1) * C, r0 : r0 + RB * GRPB, :],
                    in_=ost[:, (slot * 2 + 0) * 2048 : (slot * 2 + 1) * 2048],
                ).then_inc(s_od[slot], 16)
                eng.wait_ge(s_gp, GRPB * grp + GRPB)
                eng.dma_start(
                    out=yo_d[s * C : (s + 1) * C, r0 : r0 + RB * GRPB, :],
                    in_=ost[:, (slot * 2 + 1) * 2048 : (slot * 2 + 2) * 2048],
                ).then_inc(s_od[slot], 16)

        @blk.tensor
        def _(eng):
            # warmup: ramp the PE clock on scratch data while input DMAs land
            for i in range(26):
                eng.matmul(out=pb[4], lhsT=wup[:, 0:128], rhs=wup[:, 128:640],
                           start=True, stop=True)
            eng.wait_ge(s_w[0], 16)
            for gb in range(NB):
                s, b = gb // NBS, gb % NBS
                if gb == NBS:
                    eng.wait_ge(s_w[1], 16)
                c = _chunk_of_block(b)
                if b == 0 or c != _chunk_of_block(b - 1):
                    eng.wait_ge(s_u[s * NCH + c], 16)
                par = gb % 2
                for xi in range(XI):
                    if gb >= 2:
                        # PSUM WAR: bank par*4+xi was read during block gb-2
                        if xi == 0:
                            eng.wait_ge(s_vy, gb - 1)             # M0 freed
                        else:
                            eng.wait_ge(s_ac, 3 * (gb - 2) + xi)  # M_xi freed
                    for kh in range(KS):
                        inst = eng.matmul(
                            out=pb[par * 4 + xi],
                            lhsT=g_sb[:, gcol(s, xi, kh) : gcol(s, xi, kh) + C],
                            rhs=u_sb[:, s * HP + RB * b + kh : s * HP + RB * b + kh + RB,
                                     xi * M : (xi + 1) * M],
                            start=(kh == 0),
                            stop=(kh == KS - 1),
                        )
                    inst.then_inc(s_pe, 1)

        @blk.scalar
        def _(eng):
            for gb in range(NB):
                par = gb % 2
                if gb >= 2:
                    eng.wait_ge(s_vv, gb - 1)   # m1s/m2s[par] consumers done
                    eng.wait_ge(s_gp, gb - 1)   # m3s[par] consumer done
                eng.wait_ge(s_pe, 4 * gb + 2)
                eng.activation(m1s[:, par * 512 : par * 512 + 512],
                               pb[par * 4 + 1], COPY).then_inc(s_ac, 1)
                eng.wait_ge(s_pe, 4 * gb + 3)
                eng.activation(m2s[:, par * 512 : par * 512 + 512],
                               pb[par * 4 + 2], COPY).then_inc(s_ac, 1)
                eng.wait_ge(s_pe, 4 * gb + 4)
                eng.activation(m3s[:, par * 512 : par * 512 + 512],
                               pb[par * 4 + 3], COPY).then_inc(s_ac, 1)

        @blk.vector
        def _(eng):
            for gb in range(NB):
                par, grp, j = gb % 2, gb // GRPB, gb % GRPB
                slot = grp % NSLOT
                eng.wait_ge(s_ac, 3 * gb + 2)     # m1s, m2s ready
                if gb >= 2:
                    eng.wait_ge(s_gp, gb - 1)     # uv.v[par] consumer done
                if grp >= NSLOT and j == 0:
                    eng.wait_ge(s_od[slot], 32 * (grp // NSLOT))
                mp1 = m1s[:, par * 512 : par * 512 + 512]
                mp2 = m2s[:, par * 512 : par * 512 + 512]
                eng.tensor_tensor(uv[:, par * 1024 : par * 1024 + 512],
                                  mp1, mp2, ADD)
                eng.tensor_tensor(uv[:, par * 1024 + 512 : par * 1024 + 1024],
                                  mp1, mp2, SUB).then_inc(s_vv, 1)
                dst = (slot * 2 + 0) * 2048 + j * 512
                eng.tensor_tensor(ost[:, dst : dst + 512],
                                  pb[par * 4 + 0],
                                  uv[:, par * 1024 : par * 1024 + 512],
                                  ADD).then_inc(s_vy, 1)

        @blk.gpsimd
        def _(eng):
            for gb in range(NB):
                par, grp, j = gb % 2, gb // GRPB, gb % GRPB
                slot = grp % NSLOT
                eng.wait_ge(s_vv, gb + 1)         # v ready
                eng.wait_ge(s_ac, 3 * gb + 3)     # m3s ready
                if grp >= NSLOT and j == 0:
                    eng.wait_ge(s_od[slot], 32 * (grp // NSLOT))
                dst = (slot * 2 + 1) * 2048 + j * 512
                eng.tensor_tensor(ost[:, dst : dst + 512],
                                  uv[:, par * 1024 + 512 : par * 1024 + 1024],
                                  m3s[:, par * 512 : par * 512 + 512],
                                  SUB).then_inc(s_gp, 1)

    return nc


def _host_prep(x, w, weight, mod_w, mod_b):
    f = np.float32
    import ml_dtypes
    bf = ml_dtypes.bfloat16
    x = np.asarray(x, f)
    w = np.asarray(w, f)
    weight = np.asarray(weight, f)
    mod_w = np.asarray(mod_w, f)
    mod_b = np.asarray(mod_b, f)

    s_style = (w @ mod_w.T + mod_b) + 1.0                      # [B, C_in]
    a_sq = (weight ** 2).sum(axis=(2, 3))                      # [C_out, C_in]
    d = 1.0 / np.sqrt((s_style ** 2) @ a_sq.T + 1e-8)          # [B, C_out]

    # G-transformed demodulated weights (style folded into x instead)
    wd = weight[None] * d[:, :, None, None, None]              # [B, o, i, kh, kw]
    g0 = wd[..., 0]
    g1 = 0.5 * (wd[..., 0] + wd[..., 1] + wd[..., 2])
    g2 = 0.5 * (wd[..., 0] - wd[..., 1] + wd[..., 2])
    g3 = wd[..., 2]
    G = np.stack([g0, g1, g2, g3], axis=1)                     # [B, xi, o, i, kh]
    G = np.ascontiguousarray(G.transpose(0, 3, 1, 4, 2))       # [B, i, xi, kh, o]
    G = G.astype(bf)

    # style-modulated, padded input; even/odd columns; winograd transform
    xp = np.zeros((B, C, HP, HP), f)
    xp[:, :, 1 : H + 1, 1 : W + 1] = x * s_style[:, :, None, None]
    xe = xp[..., 0::2]
    xo = xp[..., 1::2]
    U = np.empty((B, C, HP, XI, M), f)
    U[:, :, :, 0] = xe[..., :M] - xe[..., 1:]
    U[:, :, :, 1] = xo[..., :M] + xe[..., 1:]
    U[:, :, :, 2] = xe[..., 1:] - xo[..., :M]
    U[:, :, :, 3] = xo[..., :M] - xo[..., 1:]
    U = U.astype(bf)

    in_maps = []
    for core in range(NCORES):
        s0 = SPC * core
        in_maps.append({
            "u": np.ascontiguousarray(U[s0 : s0 + SPC]).reshape(SPC * C, HP, XI * M),
            "g": np.ascontiguousarray(
                G[s0 : s0 + SPC].transpose(1, 0, 2, 3, 4)).reshape(C, SPC * 12 * C),
        })
    return in_maps


def _gather(res):
    y = np.empty((B, C, H, W), np.float32)
    for core in range(NCORES):
        ye = np.asarray(res.results[core]["ye"]).astype(np.float32).reshape(SPC, C, H, M)
        yo = np.asarray(res.results[core]["yo"]).astype(np.float32).reshape(SPC, C, H, M)
        for s in range(SPC):
            y[SPC * core + s, :, :, 0::2] = ye[s]
            y[SPC * core + s, :, :, 1::2] = yo[s]
    return y


_cached = {}


def kernel(x, w, weight, mod_w, mod_b):
    if "nc" not in _cached:
        _cached["nc"] = build_program()
    nc = _cached["nc"]
    in_maps = _host_prep(x, w, weight, mod_w, mod_b)
    res = run_bass_kernel_spmd(nc, in_maps, list(range(NCORES)))
    return _gather(res)


if __name__ == "__main__":
    from concourse.bass_utils import compile_bass_kernel
    import tempfile

    nc = build_program()
    d = tempfile.mkdtemp()
    neff = compile_bass_kernel(nc, d)
    print("compiled OK:", neff)


# revision 11
# speedup vs baseline: 1.4264x; 1.1158x over previous
"""Modulated conv2d (StyleGAN-2 style, B=16 C=128 HxW=128x128 K=3) on 8 TRN2
NeuronCores, data-parallel over batch (2 samples/core), via 1D Winograd
F(2,3) along W.

All input-side transforms are pure functions of the inputs and run on HOST:
  s[b,i]   = Linear(w)+1 (style), folded into x:  xt = s * x
  d[b,o]   = demod rsqrt(sum((weight*s)^2)+eps), folded into the weights
  U[xi]    = 1D Winograd input transform of padded xt (4 tensors, bf16):
               u0=xe[m]-xe[m+1], u1=xo[m]+xe[m+1], u2=xe[m+1]-xo[m],
               u3=xo[m]-xo[m+1]        (xe/xo = even/odd padded columns)
  g[xi,kh] = G-transformed demodulated base weight (per sample, bf16):
               g0=W0, g1=(W0+W1+W2)/2, g2=(W0-W1+W2)/2, g3=W2   (kw taps)

Device work per 8-row output block (32 blocks/core, PSUM-bank sized N=512):
  PE:     12 matmuls (4 xi-groups x 3 kh, K=C_in=128) -> M0..M3 in 4 banks
  ACT:    copy M1,M2,M3 from PSUM to SBUF (m1s,m2s,m3s)
  DVE:    u=m1s+m2s, v=m1s-m2s, ye=(M0+u) -> bf16   (even output columns)
  GPSIMD: yo=(v-m3s) -> bf16                        (odd output columns)
  Winograd identity: ye = M0+M1+M2, yo = M1-M2-M3.
Even/odd column planes DMA out as separate bf16 tensors; host interleaves.

This cuts PE streaming cycles 1.5x vs direct conv (12xN=512 per 1024
outputs vs 18xN=512): PE ~83us vs the ~125us direct-conv floor. GPSIMD
cannot read PSUM (hardware rule: max one PSUM operand per vector op), hence
the ACT copies. Weight loads (12/block) hide under the 216ns matmul streams.

Raw Bass with manual semaphores (single-wait rule; every cross-engine and
PSUM/SBUF WAR dependency guarded). Numerics: bf16 operands, fp32 PSUM
accumulation and output transform, bf16 output; rel err ~4e-3 vs fp32 ref.
"""

import sys

sys.path.insert(0, "/opt/trn_rl_repo")

import numpy as np

import concourse.bass as bass
from concourse import mybir
from concourse.bass_utils import run_bass_kernel_spmd

B, C, H, W, KS, WD = 16, 128, 128, 128, 3, 512
NCORES = 8
SPC = B // NCORES          # samples per core = 2
HP = H + 2                 # padded rows = 130
M = W // 2                 # output column pairs = 64
XI = 4                     # winograd components
RB = 8                     # output rows per block (N = RB*M = 512, one bank)
NBS = H // RB              # blocks per sample = 16
NB = SPC * NBS             # blocks per core = 32
NSLOT = 3                  # output staging slots (4 blocks each)
GRPB = 4                   # blocks per output DMA group
NGRP = NB // GRPB          # 8 output DMA groups
CHUNK_BNDS = [0, 10, 18, 34, 66, 98, 130]   # U DMA row chunks
NCH = len(CHUNK_BNDS) - 1

F32 = mybir.dt.float32
BF16 = mybir.dt.bfloat16
ADD = mybir.AluOpType.add
SUB = mybir.AluOpType.subtract
COPY = mybir.ActivationFunctionType.Copy


def _chunk_of_block(b):
    need = RB * b + RB + 1
    for c in range(NCH):
        if need < CHUNK_BNDS[c + 1]:
            return c
    raise AssertionError


def build_program():
    nc = bass.Bass(trn_type="TRN2", target_bir_lowering=False, debug=False)

    # DRAM. U row layout [c, row, xi*M]: one DMA per (sample, row-chunk).
    u_d = nc.dram_tensor("u", [SPC * C, HP, XI * M], BF16, kind="ExternalInput").ap()
    g_d = nc.dram_tensor("g", [C, SPC * 12 * C], BF16, kind="ExternalInput").ap()
    ye_d = nc.dram_tensor("ye", [SPC * C, H, M], BF16, kind="ExternalOutput").ap()
    yo_d = nc.dram_tensor("yo", [SPC * C, H, M], BF16, kind="ExternalOutput").ap()

    # SBUF (per partition: 130KB U + 6KB g + 12KB m + 8KB uv + 24KB ost)
    u_sb = nc.alloc_sbuf_tensor("u_sb", [C, SPC * HP, XI * M], BF16).ap()
    g_sb = nc.alloc_sbuf_tensor("g_sb", [C, SPC * 12 * C], BF16).ap()
    wup = nc.alloc_sbuf_tensor("wup", [C, 640], BF16).ap()  # PE warmup scratch
    m1s = nc.alloc_sbuf_tensor("m1s", [C, 2 * 512], F32).ap()
    m3s = nc.alloc_sbuf_tensor("m3s", [C, 2 * 512], F32).ap()
    uv = nc.alloc_sbuf_tensor("uv", [C, 2 * 2 * 512], F32).ap()
    ost = nc.alloc_sbuf_tensor("ost", [C, NSLOT * 2 * GRPB * 512], BF16).ap()

    pb = [nc.alloc_psum_tensor(f"pb{j}", [C, 512], F32).ap() for j in range(8)]

    s_u = [nc.alloc_semaphore(f"su{i}") for i in range(SPC * NCH)]
    s_w = [nc.alloc_semaphore(f"sw{i}") for i in range(SPC)]
    s_pe = nc.alloc_semaphore("s_pe")      # +1 per xi-group (4/block)
    s_ac = nc.alloc_semaphore("s_ac")      # +1 per ACT copy (3/block)
    s_vv = nc.alloc_semaphore("s_vv")      # +1 per DVE v
    s_vy = nc.alloc_semaphore("s_vy")      # +1 per DVE ye
    s_gp = nc.alloc_semaphore("s_gp")      # +1 per GPSIMD yo
    s_od = [nc.alloc_semaphore(f"sod{i}") for i in range(NSLOT)]

    def gcol(s, xi, kh):
        return (s * 12 + 3 * xi + kh) * C

    with nc.Block() as blk:

        @blk.sync
        def _(eng):
            def uchunk(s, ci):
                r0, r1 = CHUNK_BNDS[ci], CHUNK_BNDS[ci + 1]
                eng.dma_start(
                    out=u_sb[:, s * HP + r0 : s * HP + r1, :],
                    in_=u_d[s * C : (s + 1) * C, r0:r1, :],
                ).then_inc(s_u[s * NCH + ci], 16)

            for s in range(SPC):
                for ci in range(NCH):
                    uchunk(s, ci)

        @blk.tensor
        def _(eng):
            # warmup: ramp the PE clock on scratch data while input DMAs land
            for i in range(10):
                eng.matmul(out=pb[4], lhsT=wup[:, 0:128], rhs=wup[:, 128:640],
                           start=True, stop=True)
            eng.wait_ge(s_w[0], 16)
            for gb in range(NB):
                s, b = gb // NBS, gb % NBS
                if gb == NBS:
                    eng.wait_ge(s_w[1], 16)
                c = _chunk_of_block(b)
                if b == 0 or c != _chunk_of_block(b - 1):
                    eng.wait_ge(s_u[s * NCH + c], 16)
                par = gb % 2
                for xi in range(XI):
                    if gb >= 2:
                        # PSUM WAR: bank par*4+xi was read during block gb-2
                        if xi == 0:
                            eng.wait_ge(s_vy, gb - 1)             # M0 freed
                        elif xi == 2:
                            eng.wait_ge(s_vv, gb - 1)             # M2 freed
                        else:
                            eng.wait_ge(s_ac, 2 * (gb - 2) + (1 if xi == 1 else 2))
                    for kh in range(KS):
                        inst = eng.matmul(
                            out=pb[par * 4 + xi],
                            lhsT=g_sb[:, gcol(s, xi, kh) : gcol(s, xi, kh) + C],
                            rhs=u_sb[:, s * HP + RB * b + kh : s * HP + RB * b + kh + RB,
                                     xi * M : (xi + 1) * M],
                            start=(kh == 0),
                            stop=(kh == KS - 1),
                        )
                    inst.then_inc(s_pe, 1)

        @blk.scalar
        def _(eng):
            # weight DMAs ride ACT's load queue, overlapping the q1 U stream
            eng.dma_start(out=g_sb[:, 0 : 12 * C], in_=g_d[:, 0 : 12 * C]).then_inc(
                s_w[0], 16)
            eng.dma_start(out=g_sb[:, 12 * C :], in_=g_d[:, 12 * C :]).then_inc(
                s_w[1], 16)
            for gb in range(NB):
                par = gb % 2
                if gb >= 2:
                    eng.wait_ge(s_vv, gb - 1)   # m1s[par] consumers done
                    eng.wait_ge(s_gp, gb - 1)   # m3s[par] consumer done
                # output DMAs for finished group (gb = 4*grp+5): store queue
                if gb >= 5 and (gb - 5) % GRPB == 0:
                    grp = (gb - 5) // GRPB
                    s, r0 = grp // (NGRP // SPC), RB * GRPB * (grp % (NGRP // SPC))
                    slot = grp % NSLOT
                    eng.wait_ge(s_vy, GRPB * grp + GRPB)
                    eng.dma_start(
                        out=ye_d[s * C : (s + 1) * C, r0 : r0 + RB * GRPB, :],
                        in_=ost[:, (slot * 2 + 0) * 2048 : (slot * 2 + 1) * 2048],
                    ).then_inc(s_od[slot], 16)
                    eng.dma_start(
                        out=yo_d[s * C : (s + 1) * C, r0 : r0 + RB * GRPB, :],
                        in_=ost[:, (slot * 2 + 1) * 2048 : (slot * 2 + 2) * 2048],
                    ).then_inc(s_od[slot], 16)
                eng.wait_ge(s_pe, 4 * gb + 2)
                eng.activation(m1s[:, par * 512 : par * 512 + 512],
                               pb[par * 4 + 1], COPY).then_inc(s_ac, 1)
                eng.wait_ge(s_pe, 4 * gb + 4)
                eng.activation(m3s[:, par * 512 : par * 512 + 512],
                               pb[par * 4 + 3], COPY).then_inc(s_ac, 1)
            # tail: last group
            for grp in (NGRP - 1,):
                s, r0 = grp // (NGRP // SPC), RB * GRPB * (grp % (NGRP // SPC))
                slot = grp % NSLOT
                eng.wait_ge(s_vy, GRPB * grp + GRPB)
                eng.dma_start(
                    out=ye_d[s * C : (s + 1) * C, r0 : r0 + RB * GRPB, :],
                    in_=ost[:, (slot * 2 + 0) * 2048 : (slot * 2 + 1) * 2048],
                ).then_inc(s_od[slot], 16)
                eng.wait_ge(s_gp, GRPB * grp + GRPB)
                eng.dma_start(
                    out=yo_d[s * C : (s + 1) * C, r0 : r0 + RB * GRPB, :],
                    in_=ost[:, (slot * 2 + 1) * 2048 : (slot * 2 + 2) * 2048],
                ).then_inc(s_od[slot], 16)

        @blk.vector
        def _(eng):
            for gb in range(NB):
                par, grp, j = gb % 2, gb // GRPB, gb % GRPB
                slot = grp % NSLOT
                eng.wait_ge(s_ac, 2 * gb + 1)     # m1s ready
                eng.wait_ge(s_pe, 4 * gb + 3)     # M2 ready
                if gb >= 2:
                    eng.wait_ge(s_gp, gb - 1)     # uv.v[par] consumer done
                if grp >= NSLOT and j == 0:
                    eng.wait_ge(s_od[slot], 32 * (grp // NSLOT))
                mp1 = m1s[:, par * 512 : par * 512 + 512]
                eng.tensor_tensor(uv[:, par * 1024 : par * 1024 + 512],
                                  pb[par * 4 + 2], mp1, ADD)
                eng.tensor_tensor(uv[:, par * 1024 + 512 : par * 1024 + 1024],
                                  mp1, pb[par * 4 + 2], SUB).then_inc(s_vv, 1)
                dst = (slot * 2 + 0) * 2048 + j * 512
                eng.tensor_tensor(ost[:, dst : dst + 512],
                                  pb[par * 4 + 0],
                                  uv[:, par * 1024 : par * 1024 + 512],
                                  ADD).then_inc(s_vy, 1)

        @blk.gpsimd
        def _(eng):
            for gb in range(NB):
                par, grp, j = gb % 2, gb // GRPB, gb % GRPB
                slot = grp % NSLOT
                eng.wait_ge(s_vv, gb + 1)         # v ready
                eng.wait_ge(s_ac, 2 * gb + 2)     # m3s ready
                if grp >= NSLOT and j == 0:
                    eng.wait_ge(s_od[slot], 32 * (grp // NSLOT))
                dst = (slot * 2 + 1) * 2048 + j * 512
                eng.tensor_tensor(ost[:, dst : dst + 512],
                                  uv[:, par * 1024 + 512 : par * 1024 + 1024],
                                  m3s[:, par * 512 : par * 512 + 512],
                                  SUB).then_inc(s_gp, 1)

    return nc


def _host_prep(x, w, weight, mod_w, mod_b):
    f = np.float32
    import ml_dtypes
    bf = ml_dtypes.bfloat16
    x = np.asarray(x, f)
    w = np.asarray(w, f)
    weight = np.asarray(weight, f)
    mod_w = np.asarray(mod_w, f)
    mod_b = np.asarray(mod_b, f)

    s_style = (w @ mod_w.T + mod_b) + 1.0                      # [B, C_in]
    a_sq = (weight ** 2).sum(axis=(2, 3))                      # [C_out, C_in]
    d = 1.0 / np.sqrt((s_style ** 2) @ a_sq.T + 1e-8)          # [B, C_out]

    # G-transformed demodulated weights (style folded into x instead)
    wd = weight[None] * d[:, :, None, None, None]              # [B, o, i, kh, kw]
    g0 = wd[..., 0]
    g1 = 0.5 * (wd[..., 0] + wd[..., 1] + wd[..., 2])
    g2 = 0.5 * (wd[..., 0] - wd[..., 1] + wd[..., 2])
    g3 = wd[..., 2]
    G = np.stack([g0, g1, g2, g3], axis=1)                     # [B, xi, o, i, kh]
    G = np.ascontiguousarray(G.transpose(0, 3, 1, 4, 2))       # [B, i, xi, kh, o]
    G = G.astype(bf)

    # style-modulated, padded input; even/odd columns; winograd transform
    xp = np.zeros((B, C, HP, HP), f)
    xp[:, :, 1 : H + 1, 1 : W + 1] = x * s_style[:, :, None, None]
    xe = xp[..., 0::2]
    xo = xp[..., 1::2]
    U = np.empty((B, C, HP, XI, M), f)
    U[:, :, :, 0] = xe[..., :M] - xe[..., 1:]
    U[:, :, :, 1] = xo[..., :M] + xe[..., 1:]
    U[:, :, :, 2] = xe[..., 1:] - xo[..., :M]
    U[:, :, :, 3] = xo[..., :M] - xo[..., 1:]
    U = U.astype(bf)

    in_maps = []
    for core in range(NCORES):
        s0 = SPC * core
        in_maps.append({
            "u": np.ascontiguousarray(U[s0 : s0 + SPC]).reshape(SPC * C, HP, XI * M),
            "g": np.ascontiguousarray(
                G[s0 : s0 + SPC].transpose(1, 0, 2, 3, 4)).reshape(C, SPC * 12 * C),
        })
    return in_maps


def _gather(res):
    y = np.empty((B, C, H, W), np.float32)
    for core in range(NCORES):
        ye = np.asarray(res.results[core]["ye"]).astype(np.float32).reshape(SPC, C, H, M)
        yo = np.asarray(res.results[core]["yo"]).astype(np.float32).reshape(SPC, C, H, M)
        for s in range(SPC):
            y[SPC * core + s, :, :, 0::2] = ye[s]
            y[SPC * core + s, :, :, 1::2] = yo[s]
    return y


_cached = {}


def kernel(x, w, weight, mod_w, mod_b):
    if "nc" not in _cached:
        _cached["nc"] = build_program()
    nc = _cached["nc"]
    in_maps = _host_prep(x, w, weight, mod_w, mod_b)
    res = run_bass_kernel_spmd(nc, in_maps, list(range(NCORES)))
    return _gather(res)


if __name__ == "__main__":
    from concourse.bass_utils import compile_bass_kernel
    import tempfile

    nc = build_program()
    d = tempfile.mkdtemp()
    neff = compile_bass_kernel(nc, d)
    print("compiled OK:", neff)


# revision 15
# speedup vs baseline: 1.4361x; 1.0068x over previous
"""Modulated conv2d (StyleGAN-2 style, B=16 C=128 HxW=128x128 K=3) on 8 TRN2
NeuronCores, data-parallel over batch (2 samples/core), via 1D Winograd
F(2,3) along W.

All input-side transforms are pure functions of the inputs and run on HOST:
  s[b,i]   = Linear(w)+1 (style), folded into x:  xt = s * x
  d[b,o]   = demod rsqrt(sum((weight*s)^2)+eps), folded into the weights
  U[xi]    = 1D Winograd input transform of padded xt (4 tensors, bf16):
               u0=xe[m]-xe[m+1], u1=xo[m]+xe[m+1], u2=xe[m+1]-xo[m],
               u3=xo[m]-xo[m+1]        (xe/xo = even/odd padded columns)
  g[xi,kh] = G-transformed demodulated base weight (per sample, bf16):
               g0=W0, g1=(W0+W1+W2)/2, g2=(W0-W1+W2)/2, g3=W2   (kw taps)

Device work per 8-row output block (32 blocks/core, PSUM-bank sized N=512):
  PE:     12 matmuls (4 xi-groups x 3 kh, K=C_in=128) -> M0..M3 in 4 banks
  ACT:    copy M1,M2,M3 from PSUM to SBUF (m1s,m2s,m3s)
  DVE:    u=m1s+m2s, v=m1s-m2s, ye=(M0+u) -> bf16   (even output columns)
  GPSIMD: yo=(v-m3s) -> bf16                        (odd output columns)
  Winograd identity: ye = M0+M1+M2, yo = M1-M2-M3.
Even/odd column planes DMA out as separate bf16 tensors; host interleaves.

This cuts PE streaming cycles 1.5x vs direct conv (12xN=512 per 1024
outputs vs 18xN=512): PE ~83us vs the ~125us direct-conv floor. GPSIMD
cannot read PSUM (hardware rule: max one PSUM operand per vector op), hence
the ACT copies. Weight loads (12/block) hide under the 216ns matmul streams.

Raw Bass with manual semaphores (single-wait rule; every cross-engine and
PSUM/SBUF WAR dependency guarded). Numerics: bf16 operands, fp32 PSUM
accumulation and output transform, bf16 output; rel err ~4e-3 vs fp32 ref.
"""

import sys

sys.path.insert(0, "/opt/trn_rl_repo")

import numpy as np

import concourse.bass as bass
from concourse import mybir
from concourse.bass_utils import run_bass_kernel_spmd

B, C, H, W, KS, WD = 16, 128, 128, 128, 3, 512
NCORES = 8
SPC = B // NCORES          # samples per core = 2
HP = H + 2                 # padded rows = 130
M = W // 2                 # output column pairs = 64
XI = 4                     # winograd components
RB = 8                     # output rows per block (N = RB*M = 512, one bank)
NBS = H // RB              # blocks per sample = 16
NB = SPC * NBS             # blocks per core = 32
NSLOT = 3                  # output staging slots (4 blocks each)
GRPB = 4                   # blocks per output DMA group
NGRP = NB // GRPB          # 8 output DMA groups
CHUNK_BNDS = [0, 10, 18, 34, 66, 98, 130]   # U DMA row chunks
NCH = len(CHUNK_BNDS) - 1

F32 = mybir.dt.float32
BF16 = mybir.dt.bfloat16
ADD = mybir.AluOpType.add
SUB = mybir.AluOpType.subtract
COPY = mybir.ActivationFunctionType.Copy


def _chunk_of_block(b):
    need = RB * b + RB + 1
    for c in range(NCH):
        if need < CHUNK_BNDS[c + 1]:
            return c
    raise AssertionError


def build_program():
    nc = bass.Bass(trn_type="TRN2", target_bir_lowering=False, debug=False)

    # DRAM. U row layout [c, row, xi*M]: one DMA per (sample, row-chunk).
    u_d = nc.dram_tensor("u", [SPC * C, HP, XI * M], BF16, kind="ExternalInput").ap()
    g_d = nc.dram_tensor("g", [C, SPC * 12 * C], BF16, kind="ExternalInput").ap()
    ye_d = nc.dram_tensor("ye", [SPC * C, H, M], BF16, kind="ExternalOutput").ap()
    yo_d = nc.dram_tensor("yo", [SPC * C, H, M], BF16, kind="ExternalOutput").ap()

    # SBUF (per partition: 130KB U + 6KB g + 12KB m + 8KB uv + 24KB ost)
    u_sb = nc.alloc_sbuf_tensor("u_sb", [C, SPC * HP, XI * M], BF16).ap()
    g_sb = nc.alloc_sbuf_tensor("g_sb", [C, SPC * 12 * C], BF16).ap()
    wup = nc.alloc_sbuf_tensor("wup", [C, 640], BF16).ap()  # PE warmup scratch
    m1s = nc.alloc_sbuf_tensor("m1s", [C, 2 * 512], F32).ap()
    m3s = nc.alloc_sbuf_tensor("m3s", [C, 2 * 512], F32).ap()
    uv = nc.alloc_sbuf_tensor("uv", [C, 2 * 2 * 512], F32).ap()
    ost = nc.alloc_sbuf_tensor("ost", [C, NSLOT * 2 * GRPB * 512], BF16).ap()

    pb = [nc.alloc_psum_tensor(f"pb{j}", [C, 512], F32).ap() for j in range(8)]

    s_u = [nc.alloc_semaphore(f"su{i}") for i in range(SPC * NCH)]
    s_w = [nc.alloc_semaphore(f"sw{i}") for i in range(SPC)]
    s_pe = nc.alloc_semaphore("s_pe")      # +1 per xi-group (4/block)
    s_ac = nc.alloc_semaphore("s_ac")      # +1 per ACT copy (3/block)
    s_vv = nc.alloc_semaphore("s_vv")      # +1 per DVE v
    s_vy = nc.alloc_semaphore("s_vy")      # +1 per DVE ye
    s_gp = nc.alloc_semaphore("s_gp")      # +1 per GPSIMD yo
    s_od = [nc.alloc_semaphore(f"sod{i}") for i in range(NSLOT)]

    def gcol(s, xi, kh):
        return (s * 12 + 3 * xi + kh) * C

    with nc.Block() as blk:

        @blk.sync
        def _(eng):
            def uchunk(s, ci):
                r0, r1 = CHUNK_BNDS[ci], CHUNK_BNDS[ci + 1]
                eng.dma_start(
                    out=u_sb[:, s * HP + r0 : s * HP + r1, :],
                    in_=u_d[s * C : (s + 1) * C, r0:r1, :],
                ).then_inc(s_u[s * NCH + ci], 16)

            eng.dma_start(out=g_sb[:, 0 : 12 * C], in_=g_d[:, 0 : 12 * C]).then_inc(
                s_w[0], 16)
            for s in range(SPC):
                for ci in range(NCH):
                    uchunk(s, ci)

        @blk.tensor
        def _(eng):
            # warmup: ramp the PE clock on scratch data while input DMAs land
            for i in range(13):
                eng.matmul(out=pb[4], lhsT=wup[:, 0:128], rhs=wup[:, 128:640],
                           start=True, stop=True)
            eng.wait_ge(s_w[0], 16)
            for gb in range(NB):
                s, b = gb // NBS, gb % NBS
                if gb == NBS:
                    eng.wait_ge(s_w[1], 16)
                c = _chunk_of_block(b)
                if b == 0 or c != _chunk_of_block(b - 1):
                    eng.wait_ge(s_u[s * NCH + c], 16)
                par = gb % 2
                for xi in range(XI):
                    if gb >= 2:
                        # PSUM WAR: bank par*4+xi was read during block gb-2
                        if xi == 0:
                            eng.wait_ge(s_vy, gb - 1)             # M0 freed
                        elif xi == 2:
                            eng.wait_ge(s_vv, gb - 1)             # M2 freed
                        else:
                            eng.wait_ge(s_ac, 2 * (gb - 2) + (1 if xi == 1 else 2))
                    for kh in range(KS):
                        inst = eng.matmul(
                            out=pb[par * 4 + xi],
                            lhsT=g_sb[:, gcol(s, xi, kh) : gcol(s, xi, kh) + C],
                            rhs=u_sb[:, s * HP + RB * b + kh : s * HP + RB * b + kh + RB,
                                     xi * M : (xi + 1) * M],
                            start=(kh == 0),
                            stop=(kh == KS - 1),
                        )
                    inst.then_inc(s_pe, 1)

        @blk.scalar
        def _(eng):
            # sample-1 weights ride ACT's queue, overlapping the q1 U stream
            eng.dma_start(out=g_sb[:, 12 * C :], in_=g_d[:, 12 * C :]).then_inc(
                s_w[1], 16)
            for gb in range(NB):
                par = gb % 2
                if gb >= 2:
                    eng.wait_ge(s_vv, gb - 1)   # m1s[par] consumers done
                    eng.wait_ge(s_gp, gb - 1)   # m3s[par] consumer done
                # output DMAs for finished group (gb = 4*grp+5): store queue
                if gb >= 5 and (gb - 5) % GRPB == 0:
                    grp = (gb - 5) // GRPB
                    s, r0 = grp // (NGRP // SPC), RB * GRPB * (grp % (NGRP // SPC))
                    slot = grp % NSLOT
                    eng.wait_ge(s_vy, GRPB * grp + GRPB)
                    eng.dma_start(
                        out=ye_d[s * C : (s + 1) * C, r0 : r0 + RB * GRPB, :],
                        in_=ost[:, (slot * 2 + 0) * 2048 : (slot * 2 + 1) * 2048],
                    ).then_inc(s_od[slot], 16)
                    eng.dma_start(
                        out=yo_d[s * C : (s + 1) * C, r0 : r0 + RB * GRPB, :],
                        in_=ost[:, (slot * 2 + 1) * 2048 : (slot * 2 + 2) * 2048],
                    ).then_inc(s_od[slot], 16)
                eng.wait_ge(s_pe, 4 * gb + 2)
                eng.activation(m1s[:, par * 512 : par * 512 + 512],
                               pb[par * 4 + 1], COPY).then_inc(s_ac, 1)
                eng.wait_ge(s_pe, 4 * gb + 4)
                eng.activation(m3s[:, par * 512 : par * 512 + 512],
                               pb[par * 4 + 3], COPY).then_inc(s_ac, 1)
            # tail: last group, split so the final DMA is one block deep
            grp = NGRP - 1
            s, r0 = grp // (NGRP // SPC), RB * GRPB * (grp % (NGRP // SPC))
            slot = grp % NSLOT
            eng.wait_ge(s_vy, GRPB * grp + GRPB - 1)
            eng.dma_start(
                out=ye_d[s * C : (s + 1) * C, r0 : r0 + RB * (GRPB - 1), :],
                in_=ost[:, (slot * 2 + 0) * 2048 : (slot * 2 + 0) * 2048 + 1536],
            ).then_inc(s_od[slot], 16)
            eng.wait_ge(s_gp, GRPB * grp + GRPB - 1)
            eng.dma_start(
                out=yo_d[s * C : (s + 1) * C, r0 : r0 + RB * (GRPB - 1), :],
                in_=ost[:, (slot * 2 + 1) * 2048 : (slot * 2 + 1) * 2048 + 1536],
            ).then_inc(s_od[slot], 16)
            eng.wait_ge(s_vy, GRPB * grp + GRPB)
            eng.dma_start(
                out=ye_d[s * C : (s + 1) * C, r0 + RB * (GRPB - 1) : r0 + RB * GRPB, :],
                in_=ost[:, (slot * 2 + 0) * 2048 + 1536 : (slot * 2 + 1) * 2048],
            ).then_inc(s_od[slot], 16)
            eng.wait_ge(s_gp, GRPB * grp + GRPB)
            eng.dma_start(
                out=yo_d[s * C : (s + 1) * C, r0 + RB * (GRPB - 1) : r0 + RB * GRPB, :],
                in_=ost[:, (slot * 2 + 1) * 2048 + 1536 : (slot * 2 + 2) * 2048],
            ).then_inc(s_od[slot], 16)

        @blk.vector
        def _(eng):
            for gb in range(NB):
                par, grp, j = gb % 2, gb // GRPB, gb % GRPB
                slot = grp % NSLOT
                eng.wait_ge(s_ac, 2 * gb + 1)     # m1s ready
                eng.wait_ge(s_pe, 4 * gb + 3)     # M2 ready
                if gb >= 2:
                    eng.wait_ge(s_gp, gb - 1)     # uv.v[par] consumer done
                if grp >= NSLOT and j == 0:
                    eng.wait_ge(s_od[slot], 32 * (grp // NSLOT))
                mp1 = m1s[:, par * 512 : par * 512 + 512]
                eng.tensor_tensor(uv[:, par * 1024 : par * 1024 + 512],
                                  pb[par * 4 + 2], mp1, ADD)
                eng.tensor_tensor(uv[:, par * 1024 + 512 : par * 1024 + 1024],
                                  mp1, pb[par * 4 + 2], SUB).then_inc(s_vv, 1)
                dst = (slot * 2 + 0) * 2048 + j * 512
                eng.tensor_tensor(ost[:, dst : dst + 512],
                                  pb[par * 4 + 0],
                                  uv[:, par * 1024 : par * 1024 + 512],
                                  ADD).then_inc(s_vy, 1)

        @blk.gpsimd
        def _(eng):
            for gb in range(NB):
                par, grp, j = gb % 2, gb // GRPB, gb % GRPB
                slot = grp % NSLOT
                eng.wait_ge(s_vv, gb + 1)         # v ready
                eng.wait_ge(s_ac, 2 * gb + 2)     # m3s ready
                if grp >= NSLOT and j == 0:
                    eng.wait_ge(s_od[slot], 32 * (grp // NSLOT))
                dst = (slot * 2 + 1) * 2048 + j * 512
                eng.tensor_tensor(ost[:, dst : dst + 512],
                                  uv[:, par * 1024 + 512 : par * 1024 + 1024],
                                  m3s[:, par * 512 : par * 512 + 512],
                                  SUB).then_inc(s_gp, 1)

    return nc


def _host_prep(x, w, weight, mod_w, mod_b):
    f = np.float32
    import ml_dtypes
    bf = ml_dtypes.bfloat16
    x = np.asarray(x, f)
    w = np.asarray(w, f)
    weight = np.asarray(weight, f)
    mod_w = np.asarray(mod_w, f)
    mod_b = np.asarray(mod_b, f)

    s_style = (w @ mod_w.T + mod_b) + 1.0                      # [B, C_in]
    a_sq = (weight ** 2).sum(axis=(2, 3))                      # [C_out, C_in]
    d = 1.0 / np.sqrt((s_style ** 2) @ a_sq.T + 1e-8)          # [B, C_out]

    # G-transformed demodulated weights (style folded into x instead)
    wd = weight[None] * d[:, :, None, None, None]              # [B, o, i, kh, kw]
    g0 = wd[..., 0]
    g1 = 0.5 * (wd[..., 0] + wd[..., 1] + wd[..., 2])
    g2 = 0.5 * (wd[..., 0] - wd[..., 1] + wd[..., 2])
    g3 = wd[..., 2]
    G = np.stack([g0, g1, g2, g3], axis=1)                     # [B, xi, o, i, kh]
    G = np.ascontiguousarray(G.transpose(0, 3, 1, 4, 2))       # [B, i, xi, kh, o]
    G = G.astype(bf)

    # style-modulated, padded input; even/odd columns; winograd transform
    xp = np.zeros((B, C, HP, HP), f)
    xp[:, :, 1 : H + 1, 1 : W + 1] = x * s_style[:, :, None, None]
    xe = xp[..., 0::2]
    xo = xp[..., 1::2]
    U = np.empty((B, C, HP, XI, M), f)
    U[:, :, :, 0] = xe[..., :M] - xe[..., 1:]
    U[:, :, :, 1] = xo[..., :M] + xe[..., 1:]
    U[:, :, :, 2] = xe[..., 1:] - xo[..., :M]
    U[:, :, :, 3] = xo[..., :M] - xo[..., 1:]
    U = U.astype(bf)

    in_maps = []
    for core in range(NCORES):
        s0 = SPC * core
        in_maps.append({
            "u": np.ascontiguousarray(U[s0 : s0 + SPC]).reshape(SPC * C, HP, XI * M),
            "g": np.ascontiguousarray(
                G[s0 : s0 + SPC].transpose(1, 0, 2, 3, 4)).reshape(C, SPC * 12 * C),
        })
    return in_maps


def _gather(res):
    y = np.empty((B, C, H, W), np.float32)
    for core in range(NCORES):
        ye = np.asarray(res.results[core]["ye"]).astype(np.float32).reshape(SPC, C, H, M)
        yo = np.asarray(res.results[core]["yo"]).astype(np.float32).reshape(SPC, C, H, M)
        for s in range(SPC):
            y[SPC * core + s, :, :, 0::2] = ye[s]
            y[SPC * core + s, :, :, 1::2] = yo[s]
    return y


_cached = {}


def kernel(x, w, weight, mod_w, mod_b):
    if "nc" not in _cached:
        _cached["nc"] = build_program()
    nc = _cached["nc"]
    in_maps = _host_prep(x, w, weight, mod_w, mod_b)
    res = run_bass_kernel_spmd(nc, in_maps, list(range(NCORES)))
    return _gather(res)


if __name__ == "__main__":
    from concourse.bass_utils import compile_bass_kernel
    import tempfile

    nc = build_program()
    d = tempfile.mkdtemp()
    neff = compile_bass_kernel(nc, d)
    print("compiled OK:", neff)
